# revision 2
# baseline (speedup 1.0000x reference)
"""LightGCN 3-layer SpMM on 8 TRN2 NeuronCores.

Row-sharded edge-parallel SpMM: core c owns output rows [c*12500, (c+1)*12500).
Per layer (one SPMD launch): each core SWDGE-gathers x[col] for its edges
(col-chunked to fit int16 indices), scales by edge value on the vector engine,
and SWDGE-scatter-adds into its DRAM row slice. Rows are assigned round-robin
to tiles so no row repeats within one scatter instruction (the HW CCE add is
not atomic for duplicate indices in flight). Rare overflow edges (row degree
beyond tile count) are computed on the host and added exactly.
"""
import sys

sys.path.insert(0, "/opt/trn_rl_repo")
import numpy as np

N_NODES = 100000
DIM = 64
NCORES = 8
NLAYERS = 3
RPC = N_NODES // NCORES          # 12500 rows per core
NCHUNK = 4
CH = N_NODES // NCHUNK           # 25000 col rows per gather chunk (int16-safe)
T = 8192                         # edges per tile (= per gather/scatter instr)
S = T // 128
TPCH = 13                        # tiles per chunk (13*8192 = 106496 >= ~100K+pad)
NT = NCHUNK * TPCH               # 52 tiles per core per layer
SPARE = T                        # spare rows for padding-edge scatter targets
YEXT = RPC + SPARE

_prog_cache = {}


def _build_program():
    if "nc" in _prog_cache:
        return _prog_cache["nc"]
    from concourse import bass, bacc, tile, library_config, mybir

    f32 = mybir.dt.float32
    i16 = mybir.dt.int16
    nc = bacc.Bacc(None, target_bir_lowering=False, debug=False)
    x = nc.dram_tensor("x", [N_NODES, DIM], f32, kind="ExternalInput")
    cidx = nc.dram_tensor("cidx", [NT, 128, T // 16], i16, kind="ExternalInput")
    ridx = nc.dram_tensor("ridx", [NT, 128, T // 16], i16, kind="ExternalInput")
    vals = nc.dram_tensor("vals", [NT, 128, S, 1], f32, kind="ExternalInput")
    y = nc.dram_tensor("y", [YEXT, DIM], f32, kind="ExternalOutput")

    with tile.TileContext(nc) as tc:
        nc.gpsimd.load_library(library_config.mlp)
        with (
            tc.tile_pool(name="ip", bufs=6) as ip,
            tc.tile_pool(name="gp", bufs=4) as gp,
        ):
            t = 0
            for c in range(NCHUNK):
                xc = x[c * CH:(c + 1) * CH, :]
                for _ in range(TPCH):
                    ci = ip.tile([128, T // 16], i16)
                    ri = ip.tile([128, T // 16], i16)
                    vv = ip.tile([128, S, 1], f32)
                    nc.sync.dma_start(ci[:], cidx[t])
                    nc.sync.dma_start(ri[:], ridx[t])
                    nc.sync.dma_start(vv[:], vals[t])
                    g = gp.tile([128, S, DIM], f32)
                    # SWDGE ring holds <2048 descriptors per instruction:
                    # split each 8192-token tile into 1024-token sub-ops
                    SUB = 1024
                    NS = T // SUB          # 8
                    SS = SUB // 128        # 8 slots per sub-op
                    for i in range(NS):
                        nc.gpsimd.dma_gather(
                            g[:, i * SS:(i + 1) * SS, :], xc,
                            ci[:, i * (SUB // 16):(i + 1) * (SUB // 16)],
                            SUB, SUB, DIM,
                        )
                    ga, va = bass.broadcast_tensor_aps(g[:], vv[:])
                    nc.vector.tensor_tensor(ga, ga, va, mybir.AluOpType.mult)
                    for i in range(NS):
                        nc.gpsimd.dma_scatter_add(
                            y[:], g[:, i * SS:(i + 1) * SS, :],
                            ri[:, i * (SUB // 16):(i + 1) * (SUB // 16)],
                            SUB, SUB, DIM,
                        )
                    t += 1
    nc.compile()
    _prog_cache["nc"] = nc
    return nc


def _wrap16(a):
    # [NT, T] -> [NT, 128, T//16]: token j of tile at [j%16, j//16], x8 replicas
    nt = a.shape[0]
    w = a.reshape(nt, T // 16, 16).transpose(0, 2, 1)
    return np.ascontiguousarray(np.tile(w, (1, 8, 1)))


def _prep_core(rows, cols, vvals):
    """rows: local [0,RPC); returns (cidx, ridx, vals arrays, fixup edges)."""
    chunk = cols // CH
    order = np.lexsort((rows, chunk))
    rows, cols, vvals, chunk = rows[order], cols[order], vvals[order], chunk[order]
    # occurrence rank k within each (chunk, row) group
    key = chunk.astype(np.int64) * RPC + rows
    ne = len(key)
    starts = np.flatnonzero(np.r_[True, key[1:] != key[:-1]])
    group_id = np.cumsum(np.r_[True, key[1:] != key[:-1]]) - 1
    k = np.arange(ne) - starts[group_id]
    fix = k >= TPCH
    tile_id = chunk * TPCH + (k + rows) % TPCH
    # drop fixup edges, count per-tile occupancy
    good = ~fix
    tid = tile_id[good]
    # position within tile
    order2 = np.argsort(tid, kind="stable")
    tid_s = tid[order2]
    tstarts = np.searchsorted(tid_s, np.arange(NT))
    tcounts = np.searchsorted(tid_s, np.arange(NT), side="right") - tstarts
    # per-tile overflow beyond T also goes to fixup
    pos_in_tile = np.arange(len(tid_s)) - tstarts[tid_s]
    ovf = pos_in_tile >= T
    # build dense [NT, T] arrays
    cidx_a = np.zeros((NT, T), np.int16)
    ridx_a = (RPC + np.arange(T, dtype=np.int32))[None, :] * np.ones((NT, 1), np.int32)
    vals_a = np.zeros((NT, T), np.float32)
    gi = np.flatnonzero(good)[order2][~ovf]        # original (sorted) edge idx
    tt = tid_s[~ovf]
    pp = pos_in_tile[~ovf]
    cidx_a[tt, pp] = (cols[gi] - chunk[gi] * CH).astype(np.int16)
    ridx_a[tt, pp] = rows[gi]
    vals_a[tt, pp] = vvals[gi]
    ridx_a = ridx_a.astype(np.int16)
    # fixup edges: occurrence >= TPCH or tile overflow
    fixsel = np.zeros(ne, bool)
    fixsel[fix] = True
    if ovf.any():
        fixsel[np.flatnonzero(good)[order2][ovf]] = True
    fx = (rows[fixsel], cols[fixsel], vvals[fixsel])
    vals_w = vals_a.reshape(NT, S, 128).transpose(0, 2, 1)[..., None]
    return (
        _wrap16(cidx_a),
        _wrap16(ridx_a),
        np.ascontiguousarray(vals_w),
        fx,
    )


def _prep(adj_row, adj_col, adj_vals):
    per_core = []
    fix_r, fix_c, fix_v = [], [], []
    core = adj_row // RPC
    for c in range(NCORES):
        sel = core == c
        ci, ri, vv, (fr, fc, fv) = _prep_core(
            (adj_row[sel] - c * RPC).astype(np.int32),
            adj_col[sel].astype(np.int32),
            adj_vals[sel].astype(np.float32),
        )
        per_core.append({"cidx": ci, "ridx": ri, "vals": vv})
        fix_r.append(fr + c * RPC)
        fix_c.append(fc)
        fix_v.append(fv)
    return per_core, np.concatenate(fix_r), np.concatenate(fix_c), np.concatenate(fix_v)


def kernel(user_emb, item_emb, adj_vals, adj_row, adj_col):
    from concourse.bass_utils import run_bass_kernel_spmd

    nc = _build_program()
    per_core, fr, fc, fv = _prep(
        np.asarray(adj_row), np.asarray(adj_col), np.asarray(adj_vals)
    )
    x = np.concatenate([np.asarray(user_emb), np.asarray(item_emb)], axis=0).astype(
        np.float32
    )
    for _ in range(NLAYERS):
        in_maps = [{"x": x, **per_core[c]} for c in range(NCORES)]
        res = run_bass_kernel_spmd(nc, in_maps, core_ids=list(range(NCORES))).results
        y = np.empty((N_NODES, DIM), np.float32)
        for c in range(NCORES):
            y[c * RPC:(c + 1) * RPC] = res[c]["y"][:RPC]
        if len(fr):
            np.add.at(y, fr, fv[:, None] * x[fc])
        x = y
    return x


# revision 3
# speedup vs baseline: 1.5151x; 1.5151x over previous
"""LightGCN 3-layer SpMM on 8 TRN2 NeuronCores.

Row-sharded edge-parallel SpMM: core c owns output rows [c*12500, (c+1)*12500).
Per layer (one SPMD launch): each core SWDGE-gathers x[col] for its edges
(col-chunked to fit int16 indices), scales by edge value on the vector engine,
and SWDGE-scatter-adds into its DRAM row slice. Rows are assigned round-robin
to tiles so no row repeats within one scatter instruction (the HW CCE add is
not atomic for duplicate indices in flight). Rare overflow edges (row degree
beyond tile count) are computed on the host and added exactly.
"""
import sys

sys.path.insert(0, "/opt/trn_rl_repo")
import numpy as np

N_NODES = 100000
DIM = 64
NCORES = 8
NLAYERS = 3
RPC = N_NODES // NCORES          # 12500 rows per core
NCHUNK = 4
CH = N_NODES // NCHUNK           # 25000 col rows per gather chunk (int16-safe)
T = 8192                         # edges per tile (= per gather/scatter instr)
S = T // 128
TPCH = 13                        # tiles per chunk (13*8192 = 106496 >= ~100K+pad)
NT = NCHUNK * TPCH               # 52 tiles per core per layer
SPARE = T                        # spare rows for padding-edge scatter targets
YEXT = RPC + SPARE

_prog_cache = {}


def _build_program():
    if "nc" in _prog_cache:
        return _prog_cache["nc"]
    from concourse import bass, bacc, tile, library_config, mybir

    f32 = mybir.dt.float32
    i16 = mybir.dt.int16
    nc = bacc.Bacc(None, target_bir_lowering=False, debug=False)
    x = nc.dram_tensor("x", [N_NODES, DIM], f32, kind="ExternalInput")
    cidx = nc.dram_tensor("cidx", [NT, 128, T // 16], i16, kind="ExternalInput")
    ridx = nc.dram_tensor("ridx", [NT, 128, T // 16], i16, kind="ExternalInput")
    vals = nc.dram_tensor("vals", [NT, 128, S, 1], f32, kind="ExternalInput")
    y = nc.dram_tensor("y", [YEXT, DIM], f32, kind="ExternalOutput")

    with tile.TileContext(nc) as tc:
        nc.gpsimd.load_library(library_config.mlp)
        with (
            tc.tile_pool(name="ip", bufs=8) as ip,
            tc.tile_pool(name="gp", bufs=6) as gp,
        ):
            t = 0
            for c in range(NCHUNK):
                xc = x[c * CH:(c + 1) * CH, :]
                for _ in range(TPCH):
                    ci = ip.tile([128, T // 16], i16)
                    ri = ip.tile([128, T // 16], i16)
                    vv = ip.tile([128, S, 1], f32)
                    nc.sync.dma_start(ci[:], cidx[t])
                    nc.sync.dma_start(ri[:], ridx[t])
                    nc.sync.dma_start(vv[:], vals[t])
                    g = gp.tile([128, S, DIM], f32)
                    # SWDGE ring holds <2048 descriptors per instruction:
                    # split each 8192-token tile into 1024-token sub-ops
                    SUB = 1024
                    NS = T // SUB          # 8
                    SS = SUB // 128        # 8 slots per sub-op
                    for i in range(NS):
                        nc.gpsimd.dma_gather(
                            g[:, i * SS:(i + 1) * SS, :], xc,
                            ci[:, i * (SUB // 16):(i + 1) * (SUB // 16)],
                            SUB, SUB, DIM,
                        )
                    ga, va = bass.broadcast_tensor_aps(g[:], vv[:])
                    nc.vector.tensor_tensor(ga, ga, va, mybir.AluOpType.mult)
                    for i in range(NS):
                        nc.gpsimd.dma_scatter_add(
                            y[:], g[:, i * SS:(i + 1) * SS, :],
                            ri[:, i * (SUB // 16):(i + 1) * (SUB // 16)],
                            SUB, SUB, DIM,
                        )
                    t += 1
    nc.compile()
    _prog_cache["nc"] = nc
    return nc


def _wrap16(a):
    # [NT, T] -> [NT, 128, T//16]: token j of tile at [j%16, j//16], x8 replicas
    nt = a.shape[0]
    w = a.reshape(nt, T // 16, 16).transpose(0, 2, 1)
    return np.ascontiguousarray(np.tile(w, (1, 8, 1)))


def _prep_core(rows, cols, vvals):
    """rows: local [0,RPC); returns (cidx, ridx, vals arrays, fixup edges)."""
    chunk = cols // CH
    order = np.lexsort((rows, chunk))
    rows, cols, vvals, chunk = rows[order], cols[order], vvals[order], chunk[order]
    # occurrence rank k within each (chunk, row) group
    key = chunk.astype(np.int64) * RPC + rows
    ne = len(key)
    starts = np.flatnonzero(np.r_[True, key[1:] != key[:-1]])
    group_id = np.cumsum(np.r_[True, key[1:] != key[:-1]]) - 1
    k = np.arange(ne) - starts[group_id]
    fix = k >= TPCH
    tile_id = chunk * TPCH + (k + rows) % TPCH
    # drop fixup edges, count per-tile occupancy
    good = ~fix
    tid = tile_id[good]
    # position within tile
    order2 = np.argsort(tid, kind="stable")
    tid_s = tid[order2]
    tstarts = np.searchsorted(tid_s, np.arange(NT))
    tcounts = np.searchsorted(tid_s, np.arange(NT), side="right") - tstarts
    # per-tile overflow beyond T also goes to fixup
    pos_in_tile = np.arange(len(tid_s)) - tstarts[tid_s]
    ovf = pos_in_tile >= T
    # build dense [NT, T] arrays
    cidx_a = np.zeros((NT, T), np.int16)
    ridx_a = (RPC + np.arange(T, dtype=np.int32))[None, :] * np.ones((NT, 1), np.int32)
    vals_a = np.zeros((NT, T), np.float32)
    gi = np.flatnonzero(good)[order2][~ovf]        # original (sorted) edge idx
    tt = tid_s[~ovf]
    pp = pos_in_tile[~ovf]
    cidx_a[tt, pp] = (cols[gi] - chunk[gi] * CH).astype(np.int16)
    ridx_a[tt, pp] = rows[gi]
    vals_a[tt, pp] = vvals[gi]
    ridx_a = ridx_a.astype(np.int16)
    # fixup edges: occurrence >= TPCH or tile overflow
    fixsel = np.zeros(ne, bool)
    fixsel[fix] = True
    if ovf.any():
        fixsel[np.flatnonzero(good)[order2][ovf]] = True
    fx = (rows[fixsel], cols[fixsel], vvals[fixsel])
    vals_w = vals_a.reshape(NT, S, 128).transpose(0, 2, 1)[..., None]
    return (
        _wrap16(cidx_a),
        _wrap16(ridx_a),
        np.ascontiguousarray(vals_w),
        fx,
    )


def _prep(adj_row, adj_col, adj_vals):
    per_core = []
    fix_r, fix_c, fix_v = [], [], []
    core = adj_row // RPC
    for c in range(NCORES):
        sel = core == c
        ci, ri, vv, (fr, fc, fv) = _prep_core(
            (adj_row[sel] - c * RPC).astype(np.int32),
            adj_col[sel].astype(np.int32),
            adj_vals[sel].astype(np.float32),
        )
        per_core.append({"cidx": ci, "ridx": ri, "vals": vv})
        fix_r.append(fr + c * RPC)
        fix_c.append(fc)
        fix_v.append(fv)
    return per_core, np.concatenate(fix_r), np.concatenate(fix_c), np.concatenate(fix_v)


def kernel(user_emb, item_emb, adj_vals, adj_row, adj_col):
    from concourse.bass_utils import run_bass_kernel_spmd

    nc = _build_program()
    per_core, fr, fc, fv = _prep(
        np.asarray(adj_row), np.asarray(adj_col), np.asarray(adj_vals)
    )
    x = np.concatenate([np.asarray(user_emb), np.asarray(item_emb)], axis=0).astype(
        np.float32
    )
    for _ in range(NLAYERS):
        in_maps = [{"x": x, **per_core[c]} for c in range(NCORES)]
        res = run_bass_kernel_spmd(nc, in_maps, core_ids=list(range(NCORES))).results
        y = np.empty((N_NODES, DIM), np.float32)
        for c in range(NCORES):
            y[c * RPC:(c + 1) * RPC] = res[c]["y"][:RPC]
        if len(fr):
            np.add.at(y, fr, fv[:, None] * x[fc])
        x = y
    return x
